# revision 18
# baseline (speedup 1.0000x reference)
"""CP-factorized multi-head attention kernel for Trainium2 (8 NeuronCores).

Sharding: data-parallel over batch B=8, one batch element per core.

Math (per core, per head): S = U Tk^T (rank-64 logits, |S| <= 0.35), and
softmax is replaced by its quadratic Taylor expansion
    P = exp(S) ~= 1 + S + S^2/2        (truncation error ~1e-4 in O)
which decomposes the attention numerator into
    t1 = colsum(V)                      (exact, tiny)
    t2 = U @ G,  G = Tk^T V = (Tv^T Tk)^T Bv   (rank-64, tiny matmuls)
    t3 = (1/2) (S.^2) V                 (the only O(N^2) stream)
S and S^2 streams run as fp8e4 DoubleRow matmuls (2x PE column rate,
2x contraction density); S^2 is produced directly in fp8 by the scalar
engine's Square activation. Z = 1024 + sum_j S + sum_j S^2/2 comes from
ones-columns folded into the same matmuls. All fp8 streams are pre-scaled
to the e4m3 sweet spot; the single PSUM scale alpha=2048 cancels in the
final normalization (num * (1/Z) computed per head).

Verified numerics (numpy sim vs exact): rel err ~5.5e-3 (gate 2e-2).
"""

import sys

sys.path.insert(0, "/opt/trn_rl_repo")

import os
import numpy as np
from contextlib import ExitStack

import concourse.bass as bass
from concourse import bacc
import concourse.mybir as mybir
import concourse.tile as tile
from concourse.bass_utils import run_bass_kernel_spmd

FP32 = mybir.dt.float32
FP32R = mybir.dt.float32r
FP8 = mybir.dt.float8e4
BF16 = mybir.dt.bfloat16
SQUARE = mybir.ActivationFunctionType.Square
COPY = mybir.ActivationFunctionType.Copy
DR = mybir.MatmulPerfMode.DoubleRow
ADD = mybir.AluOpType.add
MULT = mybir.AluOpType.mult

B, N, DIM, H, HD, R = 8, 1024, 768, 12, 64, 64
NCORES = 8

# scale algebra: ck folded into Ak (x32), cu into M (x1024); alpha = 2048
CK = 32.0
CU = 1024.0
ALPHA = 2048.0
SQ_SCALE = 2.0 ** -10       # (32768 S * 2^-10)^2 = 1024 S^2 = (alpha/2) S^2
K2T_SCALE = 1.0 / 16.0      # psum 32*K2T -> k2t_sb = 2*K2T -> g = 2*G
TKSUM_SCALE = 1.0 / 16.0    # 32*tksum -> 2*tksum (g8 Z-col)
TVSUM_SCALE = ALPHA         # colsum column = alpha * colsum_true
ZCONST = ALPHA * 1024.0

LAST_EXEC_NS = None


def _build_nc():
    nc = bacc.Bacc(
        "TRN2", target_bir_lowering=False, debug=False, num_devices=NCORES
    )
    xt_d = nc.dram_tensor("xt", [DIM, N], FP32R, kind="ExternalInput")
    aqk_d = nc.dram_tensor("a_qk", [128, 768], FP32R, kind="ExternalInput")
    av_d = nc.dram_tensor("a_v", [128, 768], FP32R, kind="ExternalInput")
    m2_d = nc.dram_tensor("m2", [128, 768], FP32R, kind="ExternalInput")
    bv_d = nc.dram_tensor("bv", [64, 768], FP32R, kind="ExternalInput")
    pwt_d = nc.dram_tensor("pwt", [DIM, DIM], FP32R, kind="ExternalInput")
    bias_d = nc.dram_tensor("bias", [768], FP32, kind="ExternalInput")
    id_d = nc.dram_tensor("ident", [64, 64], FP32R, kind="ExternalInput")
    cs_d = nc.dram_tensor("cs", [128, 6], FP32, kind="ExternalInput")
    gz_d = nc.dram_tensor("gz", [32, 2], FP8, kind="ExternalInput")
    out_d = nc.dram_tensor("out", [N, DIM], FP32, kind="ExternalOutput")

    with tile.TileContext(nc) as tc, ExitStack() as ctx:
        sing = ctx.enter_context(tc.tile_pool(name="sing", bufs=1))
        psum = ctx.enter_context(tc.tile_pool(name="psum", bufs=2, space="PSUM"))
        u8p = ctx.enter_context(tc.tile_pool(name="u8p", bufs=3))
        s2p = ctx.enter_context(tc.tile_pool(name="s2p", bufs=3))
        zp = ctx.enter_context(tc.tile_pool(name="zp", bufs=2))
        rzp = ctx.enter_context(tc.tile_pool(name="rzp", bufs=2))
        accp = ctx.enter_context(tc.tile_pool(name="accp", bufs=3))
        obuf = ctx.enter_context(tc.tile_pool(name="obuf", bufs=2))

        xt_sb = [sing.tile([128, 1024], FP32R, tag=f"xt{k}", name=f"xt{k}") for k in range(6)]
        aqk_sb = sing.tile([128, 768], FP32R, tag="aqk")
        av_sb = sing.tile([128, 768], FP32R, tag="av")
        m2_sb = sing.tile([128, 768], FP32R, tag="m2")
        bv_sb = sing.tile([64, 768], FP32R, tag="bv")
        pw_sb = [sing.tile([128, 768], FP32R, tag=f"pw{k}", name=f"pw{k}") for k in range(6)]
        b_sb = sing.tile([128, 768], FP32, tag="b")
        id_sb = sing.tile([64, 64], FP32R, tag="id")

        tq2_sb = sing.tile([128, 1024], FP32R, tag="tq2")
        tkt_sb = sing.tile([64, 1024], FP32R, tag="tkt")
        tv_sb = sing.tile([64, 1024], FP32R, tag="tv")
        tk8_sb = sing.tile([32, 2, 1024], FP8, tag="tk8")
        tj_sb = sing.tile([128, 8, 128], BF16, tag="tj")
        k2t_sb = sing.tile([64, 64], FP32R, tag="k2t")
        g8_sb = [sing.tile([32, 2, 128], FP8, tag=f"g8{h}", name=f"g8{h}") for h in range(H)]
        v8_sb = [sing.tile([128, 2, H, 128], FP8, tag=f"v8{jp}", name=f"v8{jp}") for jp in range(4)]
        ot_sb = [sing.tile([128, 1024], FP32R, tag=f"ot{k}", name=f"ot{k}") for k in range(6)]
        cs_sb = sing.tile([128, 6], FP32, tag="cs")
        gz_sb = sing.tile([32, 2], FP8, tag="gz")

        # ---- input DMAs: xt on sync queue, weights on gpsimd queue ----
        nc.sync.dma_start(out=aqk_sb, in_=aqk_d[:, :])
        nc.sync.dma_start(out=av_sb, in_=av_d[:, :])
        for k in range(6):
            nc.sync.dma_start(out=xt_sb[k], in_=xt_d[k * 128:(k + 1) * 128, :])
        nc.gpsimd.dma_start(out=m2_sb, in_=m2_d[:, :])
        nc.gpsimd.dma_start(out=bv_sb, in_=bv_d[:, :])
        nc.gpsimd.dma_start(out=id_sb, in_=id_d[:, :])
        nc.gpsimd.dma_start(out=cs_sb, in_=cs_d[:, :])
        nc.gpsimd.dma_start(out=gz_sb, in_=gz_d[:, :])
        for k in range(6):
            nc.gpsimd.dma_start(out=pw_sb[k], in_=pwt_d[k * 128:(k + 1) * 128, :])
        nc.gpsimd.dma_start(
            out=b_sb, in_=bass.AP(tensor=bias_d, offset=0, ap=[[0, 128], [1, 768]])
        )

        # ---- T-phase: Tq^T (dup), 32*Tk^T (fp32r + fp8 + accum), Tv^T ----
        for lc in range(2):
            sl = slice(lc * 512, (lc + 1) * 512)
            ptqk = psum.tile([128, 512], FP32, tag="sm", name="ptqk")
            for k in range(6):
                nc.tensor.matmul(
                    ptqk, aqk_sb[:, k * 128:(k + 1) * 128], xt_sb[k][:, sl],
                    start=(k == 0), stop=(k == 5),
                )
            ptv = psum.tile([128, 512], FP32, tag="sm", name="ptv")
            for k in range(6):
                nc.tensor.matmul(
                    ptv, av_sb[:, k * 128:(k + 1) * 128], xt_sb[k][:, sl],
                    start=(k == 0), stop=(k == 5),
                )
            nc.vector.tensor_copy(tq2_sb[0:64, sl], ptqk[0:64, :])
            nc.vector.tensor_copy(tq2_sb[64:128, sl], ptqk[0:64, :])
            nc.vector.tensor_copy(tkt_sb[:, sl], ptqk[64:128, :])
            for t in range(2):
                nc.vector.tensor_copy(
                    tk8_sb[:, t, sl], ptqk[64 + 32 * t:96 + 32 * t, :]
                )
            nc.scalar.activation(out=tv_sb[:, sl], in_=ptv[0:64, :], func=COPY)

        # ---- TR-phase: j-major Tk/Tv tiles, K2T = Tv^T Tk, G, colsum ----
        for jt in range(8):
            trp = psum.tile([128, 512], FP32R, tag="sm", name="trp")
            jsl = slice(jt * 128, (jt + 1) * 128)
            nc.tensor.transpose(trp[:, 0:64], tkt_sb[:, jsl], id_sb)
            nc.tensor.transpose(trp[:, 64:128], tv_sb[:, jsl], id_sb)
            nc.vector.tensor_copy(tj_sb[:, jt, :], trp[:, 0:128])
        pk2 = psum.tile([128, 512], FP32, tag="pop", name="pk2")
        for jt in range(8):
            nc.tensor.matmul(
                pk2[0:64, 0:64], tj_sb[:, jt, 64:128], tj_sb[:, jt, 0:64],
                start=(jt == 0), stop=(jt == 7),
            )
        nc.scalar.activation(out=k2t_sb, in_=pk2[0:64, 0:64], func=COPY,
                             scale=K2T_SCALE)
        # parity layout: even head -> payload cols 0:64, Z col 64;
        #                odd head  -> Z col 32, payload cols 64:128
        for h in range(H):
            nc.gpsimd.memset(g8_sb[h][:, :, :], 0.0)
        pg = psum.tile([128, 512], FP32, tag="pop", name="pg")
        for h in range(H):
            c0 = (h % 6) * 64
            if h == 6:
                pg = psum.tile([128, 512], FP32, tag="pop", name="pg")
            nc.tensor.matmul(
                pg[0:64, c0:c0 + 64], k2t_sb,
                bv_sb[:, h * 64:(h + 1) * 64], start=True, stop=True,
            )
            voff, zcol = ((64, 32) if h % 2 else (0, 64))
            for t in range(2):
                nc.vector.tensor_copy(
                    g8_sb[h][:, t, voff:voff + 64], pg[32 * t:32 * t + 32, c0:c0 + 64]
                )
                nc.vector.tensor_copy(g8_sb[h][:, t, zcol:zcol + 1], gz_sb[:, t:t + 1])

        # ---- V-assembly: fp8 [128, 2, H, 128] per j-256, parity cols ----
        for jp in range(4):
            nc.gpsimd.memset(v8_sb[jp][:, :, :, :], 0.0)
            for h in range(H):
                zcol = 32 if h % 2 else 64
                nc.gpsimd.memset(v8_sb[jp][:, :, h, zcol:zcol + 1], 1.0)
        for lt in range(8):
            jsl = slice(lt * 128, (lt + 1) * 128)
            for c0, csz in ((0, 512), (512, 256)):
                pv = psum.tile([128, 512], FP32, tag="sm", name="pv")
                nc.tensor.matmul(
                    pv[:, 0:csz], tv_sb[:, jsl], bv_sb[:, c0:c0 + csz],
                    start=True, stop=True,
                )
                kk0, nkk = c0 // 128, csz // 128
                src = pv[:, 0:csz].rearrange("p (kk par c) -> p kk par c",
                                             par=2, c=64)
                dst = v8_sb[lt // 2][:, lt % 2, 2 * kk0:2 * (kk0 + nkk), :]
                dst = dst.rearrange("p (kk par) c -> p kk par c", par=2)
                nc.vector.tensor_copy(dst[:, :, 0, 0:64], src[:, :, 0, :])
                nc.vector.tensor_copy(dst[:, :, 1, 64:128], src[:, :, 1, :])

        # ---- attention: per i-chunk (512), head-pair, head ----
        for ic in range(2):
            isl = slice(ic * 512, (ic + 1) * 512)
            for p in range(6):
                pu = psum.tile([128, 512], FP32, tag="sm", name="pu")
                nc.tensor.matmul(
                    pu, m2_sb[:, p * 128:(p + 1) * 128], tq2_sb[:, isl],
                    start=True, stop=True,
                )
                u8 = [u8p.tile([32, 2, 512], FP8, tag="u8", name=f"u8_{hh}")
                      for hh in range(2)]
                for hh in range(2):
                    for t in range(2):
                        nc.vector.tensor_copy(
                            u8[hh][:, t, :],
                            pu[64 * hh + 32 * t:64 * hh + 32 * t + 32, :],
                        )
                for hh in range(2):
                    h = 2 * p + hh
                    kk, half = h // 2, (h % 2) * 64
                    zr = 32 if h % 2 else 64      # absolute Z row
                    pay = slice(64, 128) if h % 2 else slice(0, 64)
                    # UG partial (single-shot DoubleRow), init acc = UG + zc
                    pop = psum.tile([128, 512], FP32, tag="pop", name="pop_ug")
                    for ih in range(2):
                        nc.tensor.matmul(
                            pop[:, ih * 256:(ih + 1) * 256], g8_sb[h],
                            u8[hh][:, :, ih * 256:(ih + 1) * 256],
                            start=True, stop=True, perf_mode=DR,
                        )
                    acc = accp.tile([128, 512], FP32, tag="acc", name="acc")
                    zacc = zp.tile([1, 512], FP32, tag="zacc", name="zacc")
                    nc.vector.tensor_copy(acc[pay, :], pop[pay, :])
                    nc.vector.tensor_scalar_add(
                        zacc, pop[zr:zr + 1, :], ZCONST
                    )
                    for jp in range(4):
                        ps = psum.tile([128, 1024], FP32, tag="big", name="ps")
                        for t in range(2):
                            jt = 2 * jp + t
                            for ih in range(2):
                                nc.tensor.matmul(
                                    ps[:, t * 512 + ih * 256:t * 512 + (ih + 1) * 256],
                                    tk8_sb[:, :, jt * 128:(jt + 1) * 128],
                                    u8[hh][:, :, ih * 256:(ih + 1) * 256],
                                    start=True, stop=True, perf_mode=DR,
                                )
                        s2 = s2p.tile([128, 2, 512], FP8, tag="s2", name="s2")
                        nc.scalar.activation(
                            out=s2.rearrange("p a b -> p (a b)"), in_=ps,
                            func=SQUARE, scale=SQ_SCALE,
                        )
                        pop = psum.tile([128, 512], FP32, tag="pop", name="pop_pv")
                        for ih in range(2):
                            nc.tensor.matmul(
                                pop[:, ih * 256:(ih + 1) * 256],
                                v8_sb[jp][:, :, h, :],
                                s2[:, :, ih * 256:(ih + 1) * 256],
                                start=True, stop=True, perf_mode=DR,
                            )
                        nc.vector.tensor_tensor(
                            out=acc[pay, :], in0=acc[pay, :],
                            in1=pop[pay, :], op=ADD,
                        )
                        nc.vector.tensor_tensor(
                            out=zacc, in0=zacc,
                            in1=pop[zr:zr + 1, :], op=ADD,
                        )
                    # normalization: recip(Z row) -> bcast -> (acc+cs)*rzb
                    rz = zp.tile([1, 512], FP32, tag="rz", name="rz")
                    nc.vector.reciprocal_approx_fast(out=rz, in_=zacc)
                    rzb = rzp.tile([128, 512], FP32, tag="rzb", name="rzb")
                    nc.gpsimd.dma_start(
                        out=rzb[half:half + 64, :],
                        in_=bass.AP(tensor=rz.tensor, offset=rz.offset,
                                    ap=[[1, 1], [0, 64], [1, 512]]),
                    )
                    csum = cs_sb[half:half + 64, h // 2:h // 2 + 1]
                    nc.vector.scalar_tensor_tensor(
                        out=ot_sb[kk][half:half + 64, isl],
                        in0=acc[half:half + 64, :], scalar=csum,
                        in1=rzb[half:half + 64, :],
                        op0=ADD, op1=MULT,
                    )
            # ---- output projection for finished i-half ----
            for lt in range(4 * ic, 4 * ic + 4):
                ob = obuf.tile([128, 768], FP32, tag="ob")
                for c0, csz in ((0, 512), (512, 256)):
                    pout = psum.tile([128, 512], FP32, tag="sm", name="pout")
                    for k in range(6):
                        nc.tensor.matmul(
                            pout[:, 0:csz], ot_sb[k][:, lt * 128:(lt + 1) * 128],
                            pw_sb[k][:, c0:c0 + csz], start=(k == 0), stop=(k == 5),
                        )
                    nc.vector.tensor_add(
                        ob[:, c0:c0 + csz], pout[:, 0:csz], b_sb[:, c0:c0 + csz]
                    )
                nc.sync.dma_start(out=out_d[lt * 128:(lt + 1) * 128, :], in_=ob)

    nc.finalize()
    return nc


def _prep_shared(inputs):
    def comb(W1, W2):
        return np.ascontiguousarray(
            (np.asarray(W1, np.float32)[:, None, :]
             * np.asarray(W2, np.float32)[None, :, :]).reshape(DIM, R)
        )

    Aq = comb(inputs["W_Q1"], inputs["W_Q2"])
    Ak = CK * comb(inputs["W_K1"], inputs["W_K2"])
    Av = comb(inputs["W_V1"], inputs["W_V2"])
    a_qk = np.concatenate([Aq, Ak], axis=1)  # [768, 128]
    a_qk_r = np.ascontiguousarray(
        a_qk.reshape(6, 128, 128).transpose(1, 0, 2).reshape(128, 768)
    )
    av_pad = np.zeros((DIM, 128), np.float32)
    av_pad[:, 0:R] = Av
    a_v_r = np.ascontiguousarray(
        av_pad.reshape(6, 128, 128).transpose(1, 0, 2).reshape(128, 768)
    )
    W_Q0 = np.asarray(inputs["W_Q0"], np.float32)
    W_K0 = np.asarray(inputs["W_K0"], np.float32)
    W_V0 = np.asarray(inputs["W_V0"], np.float32)
    scale = HD ** -0.5
    m2 = np.zeros((128, 768), np.float32)
    for h in range(H):
        sl = slice(h * HD, (h + 1) * HD)
        M = CU * scale * (W_Q0[sl, :].T @ W_K0[sl, :])
        p, half = h // 2, (h % 2) * 64
        m2[half:half + 64, p * 128 + half:p * 128 + half + 64] = M
    bv = np.ascontiguousarray(W_V0.T)  # [64, 768]
    pwt = np.ascontiguousarray(np.asarray(inputs["proj_w"], np.float32).T)
    bias = np.asarray(inputs["proj_b"], np.float32)
    ident = np.eye(64, dtype=np.float32)
    return dict(a_qk=a_qk_r, a_v=a_v_r, m2=m2, bv=bv, pwt=pwt, bias=bias,
                ident=ident), Ak, Av


def _prep_batch(xb, Ak, Av, W_V0):
    """Per-batch host terms: colsum (alpha-scaled, column layout) and the
    g8 Z-column (2*tksum in fp8)."""
    import ml_dtypes
    xsum = xb.sum(0)                       # [768]
    tksum = (xsum @ Ak) / CK               # true tksum [64]
    tvsum = xsum @ Av                      # [64]
    cs = np.zeros((128, 6), np.float32)
    for h in range(H):
        Bvh = W_V0[h * HD:(h + 1) * HD, :].T   # [64 r, 64 c]
        col = ALPHA * (tvsum @ Bvh)            # [64]
        cs[(h % 2) * 64:(h % 2) * 64 + 64, h // 2] = col
    gz = (2.0 * tksum).reshape(2, 32).T.astype(ml_dtypes.float8_e4m3)
    return cs, np.ascontiguousarray(gz)


def kernel(**inputs) -> np.ndarray:
    global LAST_EXEC_NS
    x = np.asarray(inputs["x"], np.float32)
    shared, Ak, Av = _prep_shared(inputs)
    W_V0 = np.asarray(inputs["W_V0"], np.float32)
    in_maps = []
    for b in range(B):
        m = dict(shared)
        m["xt"] = np.ascontiguousarray(x[b].T)
        m["cs"], m["gz"] = _prep_batch(x[b], Ak, Av, W_V0)
        in_maps.append(m)

    nc = _build_nc()
    trace = os.environ.get("KERNEL_TRACE", "0") == "1"
    res = run_bass_kernel_spmd(nc, in_maps, core_ids=list(range(NCORES)),
                               trace=trace)
    LAST_EXEC_NS = res.exec_time_ns
    out = np.stack([res.results[i]["out"] for i in range(NCORES)], axis=0)
    return out.astype(np.float32)


# revision 22
# speedup vs baseline: 1.4994x; 1.4994x over previous
"""CP-factorized multi-head attention kernel for Trainium2 (8 NeuronCores).

Sharding: data-parallel over batch B=8, one batch element per core.
Per core, per head: S = U Tk^T (rank-64 logits, |S| <= 0.35, so exp needs
no max-subtraction), P = exp(S), O = P V / Z with Z from a ones-column
folded into V. All heavy matmuls fp32r (1 col/cycle); exp on the scalar
engine writes fp32r directly.

vs the original structure:
  - inputs DMA'd as fp32r directly (no SWDGE cast pass), xt on the sync
    queue and weights on the gpsimd queue in first-use order
  - pu packed 2 heads per matmul (block-diag M pairs, duplicated-Tq rhs)
  - the Z broadcast matmul is gone: reciprocal on the [1,512] Z row, then
    a partition-broadcast DMA, then one fused (po * rzb) vector op
  - output projection for i-half 0 is interleaved before attention of
    i-half 1, shrinking the tensor-idle tail
"""

import sys

sys.path.insert(0, "/opt/trn_rl_repo")

import os
import numpy as np
from contextlib import ExitStack

import concourse.bass as bass
from concourse import bacc
import concourse.mybir as mybir
import concourse.tile as tile
from concourse.bass_utils import run_bass_kernel_spmd

FP32 = mybir.dt.float32
FP32R = mybir.dt.float32r
EXP = mybir.ActivationFunctionType.Exp
MULT = mybir.AluOpType.mult

B, N, DIM, H, HD, R = 8, 1024, 768, 12, 64, 64
NCORES = 8

LAST_EXEC_NS = None


def _build_nc():
    nc = bacc.Bacc(
        "TRN2", target_bir_lowering=False, debug=False, num_devices=NCORES
    )
    xt_d = nc.dram_tensor("xt", [DIM, N], FP32R, kind="ExternalInput")
    aqk_d = nc.dram_tensor("a_qk", [128, 768], FP32R, kind="ExternalInput")
    av_d = nc.dram_tensor("a_v", [128, 768], FP32R, kind="ExternalInput")
    m2_d = nc.dram_tensor("m2", [128, 768], FP32R, kind="ExternalInput")
    bv_d = nc.dram_tensor("bv", [64, 768], FP32R, kind="ExternalInput")
    pwt_d = nc.dram_tensor("pwt", [DIM, DIM], FP32R, kind="ExternalInput")
    bias_d = nc.dram_tensor("bias", [768], FP32, kind="ExternalInput")
    z_d = nc.dram_tensor("zeros", [128, 1024], FP32, kind="ExternalInput")
    ov_d = nc.dram_tensor("onesv", [128, H, 1], FP32, kind="ExternalInput")
    out_d = nc.dram_tensor("out", [N, DIM], FP32, kind="ExternalOutput")

    with tile.TileContext(nc) as tc, ExitStack() as ctx:
        sing = ctx.enter_context(tc.tile_pool(name="sing", bufs=1))
        psum = ctx.enter_context(tc.tile_pool(name="psum", bufs=2, space="PSUM"))
        work = ctx.enter_context(tc.tile_pool(name="work", bufs=3))
        zp = ctx.enter_context(tc.tile_pool(name="zp", bufs=2))
        rzp = ctx.enter_context(tc.tile_pool(name="rzp", bufs=2))
        obuf = ctx.enter_context(tc.tile_pool(name="obuf", bufs=2))

        xt_sb = [sing.tile([128, 1024], FP32R, tag=f"xt{k}", name=f"xt{k}") for k in range(6)]
        aqk_sb = sing.tile([128, 768], FP32R, tag="aqk")
        av_sb = sing.tile([128, 768], FP32R, tag="av")
        m2_sb = sing.tile([128, 768], FP32R, tag="m2")
        bv_sb = sing.tile([64, 768], FP32R, tag="bv")
        pw_sb = [sing.tile([128, 768], FP32R, tag=f"pw{k}", name=f"pw{k}") for k in range(6)]
        b_sb = sing.tile([128, 768], FP32, tag="b")

        tq2_sb = sing.tile([128, 1024], FP32R, tag="tq2")
        tk_sb = sing.tile([128, 1024], FP32R, tag="tk")
        tv_sb = sing.tile([64, 1024], FP32R, tag="tv")
        v_sb = [sing.tile([128, H, 65], FP32R, tag=f"v{t}", name=f"v{t}") for t in range(8)]
        ot_sb = [sing.tile([128, 1024], FP32R, tag=f"ot{k}", name=f"ot{k}") for k in range(6)]
        # manual ring for U tiles: junk rows 64:128 are zeroed once so the
        # padded contraction rows (tk rows 64:128 = 0) never meet inf/nan
        u_sb = [sing.tile([128, 512], FP32R, tag=f"u{i}", name=f"u{i}") for i in range(4)]

        # ---- input DMAs: xt on sync queue, weights on gpsimd queue ----
        nc.sync.dma_start(out=aqk_sb, in_=aqk_d[:, :])
        nc.sync.dma_start(out=av_sb, in_=av_d[:, :])
        for k in range(6):
            nc.sync.dma_start(out=xt_sb[k], in_=xt_d[k * 128:(k + 1) * 128, :])
        nc.gpsimd.dma_start(out=m2_sb, in_=m2_d[:, :])
        nc.gpsimd.dma_start(out=bv_sb, in_=bv_d[:, :])
        for k in range(6):
            nc.gpsimd.dma_start(out=pw_sb[k], in_=pwt_d[k * 128:(k + 1) * 128, :])
        nc.gpsimd.dma_start(
            out=b_sb, in_=bass.AP(tensor=bias_d, offset=0, ap=[[0, 128], [1, 768]])
        )
        nc.gpsimd.dma_start(out=tk_sb[64:128, :], in_=z_d[0:64, :])
        for i in range(4):
            nc.gpsimd.dma_start(out=u_sb[i][64:128, :], in_=z_d[64:128, 0:512])
        for t in range(8):
            nc.gpsimd.dma_start(out=v_sb[t][:, :, 64:65], in_=ov_d[:, :, :])

        # ---- T-phase: Tq^T (duplicated), Tk^T, Tv^T ----
        for lc in range(2):
            sl = slice(lc * 512, (lc + 1) * 512)
            ptqk = psum.tile([128, 512], FP32, tag="sm", name="ptqk")
            for k in range(6):
                nc.tensor.matmul(
                    ptqk, aqk_sb[:, k * 128:(k + 1) * 128], xt_sb[k][:, sl],
                    start=(k == 0), stop=(k == 5),
                )
            ptv = psum.tile([128, 512], FP32, tag="sm", name="ptv")
            for k in range(6):
                nc.tensor.matmul(
                    ptv, av_sb[:, k * 128:(k + 1) * 128], xt_sb[k][:, sl],
                    start=(k == 0), stop=(k == 5),
                )
            nc.vector.tensor_copy(tq2_sb[0:64, sl], ptqk[0:64, :])
            nc.vector.tensor_copy(tq2_sb[64:128, sl], ptqk[0:64, :])
            nc.vector.tensor_copy(tk_sb[0:64, sl], ptqk[64:128, :])
            nc.scalar.activation(out=tv_sb[:, sl], in_=ptv[0:64, :],
                                 func=mybir.ActivationFunctionType.Copy)

        # ---- V-assembly: V[j, (h, c)] with ones column ----
        for lt in range(8):
            jsl = slice(lt * 128, (lt + 1) * 128)
            for c0, csz in ((0, 512), (512, 256)):
                pv = psum.tile([128, 512], FP32, tag="sm", name="pv")
                nc.tensor.matmul(
                    pv[:, 0:csz], tv_sb[:, jsl], bv_sb[:, c0:c0 + csz],
                    start=True, stop=True,
                )
                h0, nh = c0 // 64, csz // 64
                nc.vector.tensor_copy(
                    v_sb[lt][:, h0:h0 + nh, 0:64],
                    pv[:, 0:csz].rearrange("p (h d) -> p h d", d=64),
                )

        # ---- attention + interleaved projection ----
        for ic in range(2):
            isl = slice(ic * 512, (ic + 1) * 512)
            for p in range(6):
                pu = psum.tile([128, 512], FP32, tag="sm", name="pu")
                nc.tensor.matmul(
                    pu, m2_sb[:, p * 128:(p + 1) * 128], tq2_sb[:, isl],
                    start=True, stop=True,
                )
                ub = (ic * 6 + p) * 2
                us = [u_sb[ub % 4], u_sb[(ub + 1) % 4]]
                for hh in range(2):
                    nc.vector.tensor_copy(us[hh][0:64, :], pu[64 * hh:64 * hh + 64, :])
                for hh in range(2):
                    h = 2 * p + hh
                    kk, half = h // 2, (h % 2) * 64
                    po = psum.tile([128, 512], FP32, tag="po", name="po")
                    for jp in range(4):
                        ps = psum.tile([128, 1024], FP32, tag="big", name="ps")
                        for t in range(2):
                            jt = 2 * jp + t
                            nc.tensor.matmul(
                                ps[:, t * 512:(t + 1) * 512],
                                tk_sb[:, jt * 128:(jt + 1) * 128], us[hh],
                                start=True, stop=True,
                            )
                        pt = work.tile([128, 1024], FP32R, tag="pt")
                        nc.scalar.activation(out=pt, in_=ps, func=EXP,
                                             bias=0.0, scale=1.0)
                        for t in range(2):
                            jt = 2 * jp + t
                            nc.tensor.matmul(
                                po[0:65, :], v_sb[jt][:, h, :],
                                pt[:, t * 512:(t + 1) * 512],
                                start=(jt == 0), stop=(jt == 7),
                            )
                    # normalization: recip Z row, bcast via DMA, fused mul
                    zrow = zp.tile([1, 512], FP32, tag="zrow", name="zrow")
                    nc.vector.tensor_copy(zrow, po[64:65, :])
                    rz = zp.tile([1, 512], FP32, tag="rz", name="rz")
                    nc.vector.reciprocal_approx_fast(out=rz, in_=zrow)
                    rzb = rzp.tile([128, 512], FP32, tag="rzb", name="rzb")
                    nc.gpsimd.dma_start(
                        out=rzb[half:half + 64, :],
                        in_=bass.AP(tensor=rz.tensor, offset=rz.offset,
                                    ap=[[1, 1], [0, 64], [1, 512]]),
                    )
                    nc.vector.tensor_tensor(
                        out=ot_sb[kk][half:half + 64, isl],
                        in0=po[0:64, :], in1=rzb[half:half + 64, :], op=MULT,
                    )
            # ---- output projection for the finished i-half ----
            for lt in range(4 * ic, 4 * ic + 4):
                ob = obuf.tile([128, 768], FP32, tag="ob")
                for c0, csz in ((0, 512), (512, 256)):
                    pout = psum.tile([128, 512], FP32, tag="sm", name="pout")
                    for k in range(6):
                        nc.tensor.matmul(
                            pout[:, 0:csz], ot_sb[k][:, lt * 128:(lt + 1) * 128],
                            pw_sb[k][:, c0:c0 + csz], start=(k == 0), stop=(k == 5),
                        )
                    nc.vector.tensor_add(
                        ob[:, c0:c0 + csz], pout[:, 0:csz], b_sb[:, c0:c0 + csz]
                    )
                nc.sync.dma_start(out=out_d[lt * 128:(lt + 1) * 128, :], in_=ob)

    nc.finalize()
    return nc


def _prep_shared(inputs):
    def comb(W1, W2):
        return np.ascontiguousarray(
            (np.asarray(W1, np.float32)[:, None, :]
             * np.asarray(W2, np.float32)[None, :, :]).reshape(DIM, R)
        )

    Aq = comb(inputs["W_Q1"], inputs["W_Q2"])
    Ak = comb(inputs["W_K1"], inputs["W_K2"])
    Av = comb(inputs["W_V1"], inputs["W_V2"])
    a_qk = np.concatenate([Aq, Ak], axis=1)  # [768, 128]
    a_qk_r = np.ascontiguousarray(
        a_qk.reshape(6, 128, 128).transpose(1, 0, 2).reshape(128, 768)
    )
    av_pad = np.zeros((DIM, 128), np.float32)
    av_pad[:, 0:R] = Av
    a_v_r = np.ascontiguousarray(
        av_pad.reshape(6, 128, 128).transpose(1, 0, 2).reshape(128, 768)
    )
    W_Q0 = np.asarray(inputs["W_Q0"], np.float32)
    W_K0 = np.asarray(inputs["W_K0"], np.float32)
    W_V0 = np.asarray(inputs["W_V0"], np.float32)
    scale = HD ** -0.5
    m2 = np.zeros((128, 768), np.float32)
    for h in range(H):
        sl = slice(h * HD, (h + 1) * HD)
        M = scale * (W_Q0[sl, :].T @ W_K0[sl, :])
        pp, half = h // 2, (h % 2) * 64
        m2[half:half + 64, pp * 128 + half:pp * 128 + half + 64] = M
    bv = np.ascontiguousarray(W_V0.T)  # [64, 768]
    pwt = np.ascontiguousarray(np.asarray(inputs["proj_w"], np.float32).T)
    bias = np.asarray(inputs["proj_b"], np.float32)
    zeros = np.zeros((128, 1024), np.float32)
    onesv = np.ones((128, H, 1), np.float32)
    return dict(a_qk=a_qk_r, a_v=a_v_r, m2=m2, bv=bv, pwt=pwt, bias=bias,
                zeros=zeros, onesv=onesv)


def kernel(**inputs) -> np.ndarray:
    global LAST_EXEC_NS
    x = np.asarray(inputs["x"], np.float32)
    shared = _prep_shared(inputs)
    in_maps = []
    for b in range(B):
        m = dict(shared)
        m["xt"] = np.ascontiguousarray(x[b].T)
        in_maps.append(m)

    nc = _build_nc()
    trace = os.environ.get("KERNEL_TRACE", "0") == "1"
    res = run_bass_kernel_spmd(nc, in_maps, core_ids=list(range(NCORES)),
                               trace=trace)
    LAST_EXEC_NS = res.exec_time_ns
    out = np.stack([res.results[i]["out"] for i in range(NCORES)], axis=0)
    return out.astype(np.float32)


# revision 23
# speedup vs baseline: 1.6068x; 1.0716x over previous
"""CP-factorized multi-head attention kernel for Trainium2 (8 NeuronCores).

Sharding: data-parallel over batch B=8, one batch element per core.
Per core, per head: S = U Tk^T (rank-64 logits, |S| <= 0.35, so exp needs
no max-subtraction), P = exp(S), O = P V / Z with Z from a ones-column
folded into V. All heavy matmuls fp32r (1 col/cycle); exp on the scalar
engine writes fp32r directly.

vs the original structure:
  - inputs DMA'd as fp32r directly (no SWDGE cast pass), xt on the sync
    queue and weights on the gpsimd queue in first-use order
  - pu packed 2 heads per matmul (block-diag M pairs, duplicated-Tq rhs)
  - the Z broadcast matmul is gone: reciprocal on the [1,512] Z row, then
    a partition-broadcast DMA, then one fused (po * rzb) vector op
  - output projection for i-half 0 is interleaved before attention of
    i-half 1, shrinking the tensor-idle tail
"""

import sys

sys.path.insert(0, "/opt/trn_rl_repo")

import os
import numpy as np
from contextlib import ExitStack

import concourse.bass as bass
from concourse import bacc
import concourse.mybir as mybir
import concourse.tile as tile
from concourse.bass_utils import run_bass_kernel_spmd

FP32 = mybir.dt.float32
FP32R = mybir.dt.float32r
EXP = mybir.ActivationFunctionType.Exp
MULT = mybir.AluOpType.mult

B, N, DIM, H, HD, R = 8, 1024, 768, 12, 64, 64
NCORES = 8

LAST_EXEC_NS = None


def _build_nc():
    nc = bacc.Bacc(
        "TRN2", target_bir_lowering=False, debug=False, num_devices=NCORES
    )
    xt_d = nc.dram_tensor("xt", [DIM, N], FP32R, kind="ExternalInput")
    aqk_d = nc.dram_tensor("a_qk", [128, 768], FP32R, kind="ExternalInput")
    av_d = nc.dram_tensor("a_v", [128, 768], FP32R, kind="ExternalInput")
    m2_d = nc.dram_tensor("m2", [128, 768], FP32R, kind="ExternalInput")
    bv_d = nc.dram_tensor("bv", [64, 768], FP32R, kind="ExternalInput")
    pwt_d = nc.dram_tensor("pwt", [DIM, DIM], FP32R, kind="ExternalInput")
    bias_d = nc.dram_tensor("bias", [768], FP32, kind="ExternalInput")
    z_d = nc.dram_tensor("zeros", [128, 1024], FP32, kind="ExternalInput")
    ov_d = nc.dram_tensor("onesv", [128, H, 1], FP32, kind="ExternalInput")
    out_d = nc.dram_tensor("out", [N, DIM], FP32, kind="ExternalOutput")

    with tile.TileContext(nc) as tc, ExitStack() as ctx:
        sing = ctx.enter_context(tc.tile_pool(name="sing", bufs=1))
        psum = ctx.enter_context(tc.tile_pool(name="psum", bufs=2, space="PSUM"))
        work = ctx.enter_context(tc.tile_pool(name="work", bufs=3))
        zp = ctx.enter_context(tc.tile_pool(name="zp", bufs=2))
        rzp = ctx.enter_context(tc.tile_pool(name="rzp", bufs=2))
        ocp = ctx.enter_context(tc.tile_pool(name="ocp", bufs=3))
        obuf = ctx.enter_context(tc.tile_pool(name="obuf", bufs=2))

        xt_sb = [sing.tile([128, 1024], FP32R, tag=f"xt{k}", name=f"xt{k}") for k in range(6)]
        aqk_sb = sing.tile([128, 768], FP32R, tag="aqk")
        av_sb = sing.tile([128, 768], FP32R, tag="av")
        m2_sb = sing.tile([128, 768], FP32R, tag="m2")
        bv_sb = sing.tile([64, 768], FP32R, tag="bv")
        pw_sb = [sing.tile([128, 768], FP32R, tag=f"pw{k}", name=f"pw{k}") for k in range(6)]
        b_sb = sing.tile([128, 768], FP32, tag="b")

        tq2_sb = sing.tile([128, 1024], FP32R, tag="tq2")
        tk_sb = sing.tile([128, 1024], FP32R, tag="tk")
        tv_sb = sing.tile([64, 1024], FP32R, tag="tv")
        v_sb = [sing.tile([128, H, 65], FP32R, tag=f"v{t}", name=f"v{t}") for t in range(8)]
        ot_sb = [sing.tile([128, 1024], FP32R, tag=f"ot{k}", name=f"ot{k}") for k in range(6)]
        # manual ring for U tiles: junk rows 64:128 are zeroed once so the
        # padded contraction rows (tk rows 64:128 = 0) never meet inf/nan
        u_sb = [sing.tile([128, 512], FP32R, tag=f"u{i}", name=f"u{i}") for i in range(4)]

        # ---- input DMAs: xt on sync queue, weights on gpsimd queue ----
        nc.sync.dma_start(out=aqk_sb, in_=aqk_d[:, :])
        nc.sync.dma_start(out=av_sb, in_=av_d[:, :])
        for k in range(6):
            nc.sync.dma_start(out=xt_sb[k], in_=xt_d[k * 128:(k + 1) * 128, :])
        nc.gpsimd.dma_start(out=m2_sb, in_=m2_d[:, :])
        nc.gpsimd.dma_start(out=bv_sb, in_=bv_d[:, :])
        nc.gpsimd.dma_start(out=tk_sb[64:128, :], in_=z_d[0:64, :])
        for i in range(4):
            nc.gpsimd.dma_start(out=u_sb[i][64:128, :], in_=z_d[64:128, 0:512])
        for t in range(8):
            nc.gpsimd.dma_start(out=v_sb[t][:, :, 64:65], in_=ov_d[:, :, :])
        for k in range(6):
            nc.gpsimd.dma_start(out=pw_sb[k], in_=pwt_d[k * 128:(k + 1) * 128, :])
        nc.gpsimd.dma_start(
            out=b_sb, in_=bass.AP(tensor=bias_d, offset=0, ap=[[0, 128], [1, 768]])
        )

        # ---- T-phase: Tq^T (duplicated), Tk^T, Tv^T ----
        for lc in range(2):
            sl = slice(lc * 512, (lc + 1) * 512)
            ptqk = psum.tile([128, 512], FP32, tag="sm", name="ptqk")
            for k in range(6):
                nc.tensor.matmul(
                    ptqk, aqk_sb[:, k * 128:(k + 1) * 128], xt_sb[k][:, sl],
                    start=(k == 0), stop=(k == 5),
                )
            ptv = psum.tile([128, 512], FP32, tag="sm", name="ptv")
            for k in range(6):
                nc.tensor.matmul(
                    ptv, av_sb[:, k * 128:(k + 1) * 128], xt_sb[k][:, sl],
                    start=(k == 0), stop=(k == 5),
                )
            nc.vector.tensor_copy(tq2_sb[0:64, sl], ptqk[0:64, :])
            nc.vector.tensor_copy(tq2_sb[64:128, sl], ptqk[0:64, :])
            nc.vector.tensor_copy(tk_sb[0:64, sl], ptqk[64:128, :])
            nc.scalar.activation(out=tv_sb[:, sl], in_=ptv[0:64, :],
                                 func=mybir.ActivationFunctionType.Copy)

        # ---- V-assembly: V[j, (h, c)] with ones column ----
        for lt in range(8):
            jsl = slice(lt * 128, (lt + 1) * 128)
            for c0, csz in ((0, 512), (512, 256)):
                pv = psum.tile([128, 512], FP32, tag="sm", name="pv")
                nc.tensor.matmul(
                    pv[:, 0:csz], tv_sb[:, jsl], bv_sb[:, c0:c0 + csz],
                    start=True, stop=True,
                )
                h0, nh = c0 // 64, csz // 64
                nc.vector.tensor_copy(
                    v_sb[lt][:, h0:h0 + nh, 0:64],
                    pv[:, 0:csz].rearrange("p (h d) -> p h d", d=64),
                )

        def _emit_proj(lt):
            ob = obuf.tile([128, 768], FP32, tag="ob")
            for c0, csz in ((0, 512), (512, 256)):
                pout = psum.tile([128, 512], FP32, tag="sm", name="pout")
                for k in range(6):
                    nc.tensor.matmul(
                        pout[:, 0:csz], ot_sb[k][:, lt * 128:(lt + 1) * 128],
                        pw_sb[k][:, c0:c0 + csz], start=(k == 0), stop=(k == 5),
                    )
                nc.vector.tensor_add(
                    ob[:, c0:c0 + csz], pout[:, 0:csz], b_sb[:, c0:c0 + csz]
                )
            nc.sync.dma_start(out=out_d[lt * 128:(lt + 1) * 128, :], in_=ob)

        # ---- attention + interleaved projection ----
        for ic in range(2):
            isl = slice(ic * 512, (ic + 1) * 512)
            for p in range(6):
                pu = psum.tile([128, 512], FP32, tag="sm", name="pu")
                nc.tensor.matmul(
                    pu, m2_sb[:, p * 128:(p + 1) * 128], tq2_sb[:, isl],
                    start=True, stop=True,
                )
                ub = (ic * 6 + p) * 2
                us = [u_sb[ub % 4], u_sb[(ub + 1) % 4]]
                for hh in range(2):
                    nc.vector.tensor_copy(us[hh][0:64, :], pu[64 * hh:64 * hh + 64, :])
                if ic == 1 and 1 <= p <= 4:
                    _emit_proj(p - 1)
                for hh in range(2):
                    h = 2 * p + hh
                    kk, half = h // 2, (h % 2) * 64
                    po = psum.tile([128, 512], FP32, tag="po", name="po")
                    for jp in range(4):
                        ps = psum.tile([128, 1024], FP32, tag="big", name="ps")
                        for t in range(2):
                            jt = 2 * jp + t
                            nc.tensor.matmul(
                                ps[:, t * 512:(t + 1) * 512],
                                tk_sb[:, jt * 128:(jt + 1) * 128], us[hh],
                                start=True, stop=True,
                            )
                        pt = work.tile([128, 1024], FP32R, tag="pt")
                        nc.scalar.activation(out=pt, in_=ps, func=EXP,
                                             bias=0.0, scale=1.0)
                        for t in range(2):
                            jt = 2 * jp + t
                            nc.tensor.matmul(
                                po[0:65, :], v_sb[jt][:, h, :],
                                pt[:, t * 512:(t + 1) * 512],
                                start=(jt == 0), stop=(jt == 7),
                            )
                    # copy po out fast (frees the PSUM bank), then
                    # normalization: recip Z row, bcast via DMA, fused mul
                    oc = ocp.tile([128, 512], FP32, tag="oc", name="oc")
                    nc.vector.tensor_copy(oc[half:half + 64, :], po[0:64, :])
                    zrow = zp.tile([1, 512], FP32, tag="zrow", name="zrow")
                    nc.vector.tensor_copy(zrow, po[64:65, :])
                    rz = zp.tile([1, 512], FP32, tag="rz", name="rz")
                    nc.vector.reciprocal_approx_fast(out=rz, in_=zrow)
                    rzb = rzp.tile([128, 512], FP32, tag="rzb", name="rzb")
                    nc.gpsimd.dma_start(
                        out=rzb[half:half + 64, :],
                        in_=bass.AP(tensor=rz.tensor, offset=rz.offset,
                                    ap=[[1, 1], [0, 64], [1, 512]]),
                    )
                    nc.vector.tensor_tensor(
                        out=ot_sb[kk][half:half + 64, isl],
                        in0=oc[half:half + 64, :],
                        in1=rzb[half:half + 64, :], op=MULT,
                    )
        for lt in range(4, 8):
            _emit_proj(lt)

    nc.finalize()
    return nc


def _prep_shared(inputs):
    def comb(W1, W2):
        return np.ascontiguousarray(
            (np.asarray(W1, np.float32)[:, None, :]
             * np.asarray(W2, np.float32)[None, :, :]).reshape(DIM, R)
        )

    Aq = comb(inputs["W_Q1"], inputs["W_Q2"])
    Ak = comb(inputs["W_K1"], inputs["W_K2"])
    Av = comb(inputs["W_V1"], inputs["W_V2"])
    a_qk = np.concatenate([Aq, Ak], axis=1)  # [768, 128]
    a_qk_r = np.ascontiguousarray(
        a_qk.reshape(6, 128, 128).transpose(1, 0, 2).reshape(128, 768)
    )
    av_pad = np.zeros((DIM, 128), np.float32)
    av_pad[:, 0:R] = Av
    a_v_r = np.ascontiguousarray(
        av_pad.reshape(6, 128, 128).transpose(1, 0, 2).reshape(128, 768)
    )
    W_Q0 = np.asarray(inputs["W_Q0"], np.float32)
    W_K0 = np.asarray(inputs["W_K0"], np.float32)
    W_V0 = np.asarray(inputs["W_V0"], np.float32)
    scale = HD ** -0.5
    m2 = np.zeros((128, 768), np.float32)
    for h in range(H):
        sl = slice(h * HD, (h + 1) * HD)
        M = scale * (W_Q0[sl, :].T @ W_K0[sl, :])
        pp, half = h // 2, (h % 2) * 64
        m2[half:half + 64, pp * 128 + half:pp * 128 + half + 64] = M
    bv = np.ascontiguousarray(W_V0.T)  # [64, 768]
    pwt = np.ascontiguousarray(np.asarray(inputs["proj_w"], np.float32).T)
    bias = np.asarray(inputs["proj_b"], np.float32)
    zeros = np.zeros((128, 1024), np.float32)
    onesv = np.ones((128, H, 1), np.float32)
    return dict(a_qk=a_qk_r, a_v=a_v_r, m2=m2, bv=bv, pwt=pwt, bias=bias,
                zeros=zeros, onesv=onesv)


def kernel(**inputs) -> np.ndarray:
    global LAST_EXEC_NS
    x = np.asarray(inputs["x"], np.float32)
    shared = _prep_shared(inputs)
    in_maps = []
    for b in range(B):
        m = dict(shared)
        m["xt"] = np.ascontiguousarray(x[b].T)
        in_maps.append(m)

    nc = _build_nc()
    trace = os.environ.get("KERNEL_TRACE", "0") == "1"
    res = run_bass_kernel_spmd(nc, in_maps, core_ids=list(range(NCORES)),
                               trace=trace)
    LAST_EXEC_NS = res.exec_time_ns
    out = np.stack([res.results[i]["out"] for i in range(NCORES)], axis=0)
    return out.astype(np.float32)


# revision 28
# speedup vs baseline: 2.0072x; 1.2492x over previous
"""CP-factorized multi-head attention kernel for Trainium2 (8 NeuronCores).

Sharding: data-parallel over batch B=8, one batch element per core.
Per core, per head: S = U Tk^T (rank-64 logits, |S| <= 0.35, so exp needs
no max-subtraction), P = exp(S), O = P V / Z with Z from a ones-column
folded into V. All heavy matmuls fp32r (1 col/cycle); exp on the scalar
engine writes fp32r directly.

vs the original structure:
  - inputs DMA'd as fp32r directly (no SWDGE cast pass), xt on the sync
    queue and weights on the gpsimd queue in first-use order
  - pu packed 2 heads per matmul (block-diag M pairs, duplicated-Tq rhs)
  - the Z broadcast matmul is gone: reciprocal on the [1,512] Z row, then
    a partition-broadcast DMA, then one fused (po * rzb) vector op
  - output projection for i-half 0 is interleaved before attention of
    i-half 1, shrinking the tensor-idle tail
"""

import sys

sys.path.insert(0, "/opt/trn_rl_repo")

import os
import numpy as np
from contextlib import ExitStack

import concourse.bass as bass
from concourse import bacc
import concourse.mybir as mybir
import concourse.tile as tile
from concourse.bass_utils import run_bass_kernel_spmd

FP32 = mybir.dt.float32
FP32R = mybir.dt.float32r
EXP = mybir.ActivationFunctionType.Exp
MULT = mybir.AluOpType.mult

B, N, DIM, H, HD, R = 8, 1024, 768, 12, 64, 64
NCORES = 8

LAST_EXEC_NS = None


def _build_nc():
    nc = bacc.Bacc(
        "TRN2", target_bir_lowering=False, debug=False, num_devices=NCORES
    )
    xt_d = nc.dram_tensor("xt", [DIM, N], FP32R, kind="ExternalInput")
    aqk_d = nc.dram_tensor("a_qk", [128, 768], FP32R, kind="ExternalInput")
    av_d = nc.dram_tensor("a_v", [128, 768], FP32R, kind="ExternalInput")
    m2_d = nc.dram_tensor("m2", [128, 768], FP32R, kind="ExternalInput")
    bv_d = nc.dram_tensor("bv", [64, 768], FP32R, kind="ExternalInput")
    pwt_d = nc.dram_tensor("pwt", [DIM, DIM], FP32R, kind="ExternalInput")
    bias_d = nc.dram_tensor("bias", [768], FP32, kind="ExternalInput")
    z_d = nc.dram_tensor("zeros", [128, 1024], FP32, kind="ExternalInput")
    ov_d = nc.dram_tensor("onesv", [128, H, 1], FP32, kind="ExternalInput")
    oc_d = nc.dram_tensor("onescol", [128, 64], FP32, kind="ExternalInput")
    out_d = nc.dram_tensor("out", [N, DIM], FP32, kind="ExternalOutput")

    with tile.TileContext(nc) as tc, ExitStack() as ctx:
        sing = ctx.enter_context(tc.tile_pool(name="sing", bufs=1))
        psum = ctx.enter_context(tc.tile_pool(name="psum", bufs=2, space="PSUM"))
        work = ctx.enter_context(tc.tile_pool(name="work", bufs=3))
        zp = ctx.enter_context(tc.tile_pool(name="zp", bufs=2))
        rzp = ctx.enter_context(tc.tile_pool(name="rzp", bufs=2))
        ocp = ctx.enter_context(tc.tile_pool(name="ocp", bufs=3))
        obuf = ctx.enter_context(tc.tile_pool(name="obuf", bufs=2))

        xt_sb = [sing.tile([128, 1024], FP32R, tag=f"xt{k}", name=f"xt{k}") for k in range(6)]
        aqk_sb = sing.tile([128, 768], FP32R, tag="aqk")
        av_sb = sing.tile([128, 768], FP32R, tag="av")
        m2_sb = sing.tile([128, 768], FP32R, tag="m2")
        bv_sb = sing.tile([64, 768], FP32R, tag="bv")
        pw_sb = [sing.tile([128, 768], FP32R, tag=f"pw{k}", name=f"pw{k}") for k in range(6)]
        b_sb = sing.tile([128, 768], FP32, tag="b")

        tq2_sb = sing.tile([128, 1024], FP32R, tag="tq2")
        tk_sb = sing.tile([128, 1024], FP32R, tag="tk")
        tv_sb = sing.tile([64, 1024], FP32R, tag="tv")
        v_sb = [sing.tile([128, H, 65], FP32R, tag=f"v{t}", name=f"v{t}") for t in range(8)]
        ot_sb = [sing.tile([128, 1024], FP32R, tag=f"ot{k}", name=f"ot{k}") for k in range(6)]
        # manual ring for U tiles: junk rows 64:128 are zeroed once so the
        # padded contraction rows (tk rows 64:128 = 0) never meet inf/nan
        u_sb = [sing.tile([128, 512], FP32R, tag=f"u{i}", name=f"u{i}") for i in range(4)]
        ocol_sb = sing.tile([128, 64], FP32R, tag="ocol")
        rz_sb = [sing.tile([128, 512], FP32R, tag=f"rz{i}", name=f"rz{i}") for i in range(2)]

        # ---- input DMAs: xt on sync queue, weights on gpsimd queue ----
        nc.sync.dma_start(out=aqk_sb, in_=aqk_d[:, :])
        nc.sync.dma_start(out=av_sb, in_=av_d[:, :])
        for k in range(6):
            nc.sync.dma_start(out=xt_sb[k], in_=xt_d[k * 128:(k + 1) * 128, :])
        nc.gpsimd.dma_start(out=m2_sb, in_=m2_d[:, :])
        nc.gpsimd.dma_start(out=bv_sb, in_=bv_d[:, :])
        nc.gpsimd.dma_start(out=tk_sb[64:128, :], in_=z_d[0:64, :])
        for i in range(4):
            nc.gpsimd.dma_start(out=u_sb[i][64:128, :], in_=z_d[64:128, 0:512])
        for t in range(8):
            nc.gpsimd.dma_start(out=v_sb[t][:, :, 64:65], in_=ov_d[:, :, :])
        nc.gpsimd.dma_start(out=ocol_sb, in_=oc_d[:, :])
        for i in range(2):
            nc.gpsimd.dma_start(out=rz_sb[i][1:128, :], in_=z_d[0:127, 0:512])
        for k in range(6):
            nc.gpsimd.dma_start(out=pw_sb[k], in_=pwt_d[k * 128:(k + 1) * 128, :])
        nc.gpsimd.dma_start(
            out=b_sb, in_=bass.AP(tensor=bias_d, offset=0, ap=[[0, 128], [1, 768]])
        )

        # ---- T-phase: Tq^T (duplicated), Tk^T, Tv^T ----
        for lc in range(2):
            sl = slice(lc * 512, (lc + 1) * 512)
            ptqk = psum.tile([128, 512], FP32, tag="sm", name="ptqk")
            for k in range(6):
                nc.tensor.matmul(
                    ptqk, aqk_sb[:, k * 128:(k + 1) * 128], xt_sb[k][:, sl],
                    start=(k == 0), stop=(k == 5),
                )
            ptv = psum.tile([128, 512], FP32, tag="sm", name="ptv")
            for k in range(6):
                nc.tensor.matmul(
                    ptv, av_sb[:, k * 128:(k + 1) * 128], xt_sb[k][:, sl],
                    start=(k == 0), stop=(k == 5),
                )
            nc.vector.tensor_copy(tq2_sb[0:64, sl], ptqk[0:64, :])
            nc.vector.tensor_copy(tq2_sb[64:128, sl], ptqk[0:64, :])
            nc.vector.tensor_copy(tk_sb[0:64, sl], ptqk[64:128, :])
            nc.scalar.activation(out=tv_sb[:, sl], in_=ptv[0:64, :],
                                 func=mybir.ActivationFunctionType.Copy)

        # ---- V-assembly: V[j, (h, c)] with ones column ----
        for lt in range(8):
            jsl = slice(lt * 128, (lt + 1) * 128)
            for c0, csz in ((0, 512), (512, 256)):
                pv = psum.tile([128, 512], FP32, tag="sm", name="pv")
                nc.tensor.matmul(
                    pv[:, 0:csz], tv_sb[:, jsl], bv_sb[:, c0:c0 + csz],
                    start=True, stop=True,
                )
                h0, nh = c0 // 64, csz // 64
                nc.vector.tensor_copy(
                    v_sb[lt][:, h0:h0 + nh, 0:64],
                    pv[:, 0:csz].rearrange("p (h d) -> p h d", d=64),
                )

        def _emit_norm(oc, rz, kk, half, isl):
            pz = psum.tile([128, 512], FP32, tag="sm", name="pz")
            nc.tensor.matmul(pz[0:64, :], ocol_sb, rz, start=True, stop=True)
            nc.vector.tensor_tensor(
                out=ot_sb[kk][half:half + 64, isl],
                in0=oc[half:half + 64, :], in1=pz[0:64, :], op=MULT,
            )

        def _emit_proj(lt):
            ob = obuf.tile([128, 768], FP32, tag="ob")
            for c0, csz in ((0, 512), (512, 256)):
                pout = psum.tile([128, 512], FP32, tag="sm", name="pout")
                for k in range(6):
                    nc.tensor.matmul(
                        pout[:, 0:csz], ot_sb[k][:, lt * 128:(lt + 1) * 128],
                        pw_sb[k][:, c0:c0 + csz], start=(k == 0), stop=(k == 5),
                    )
                nc.vector.tensor_add(
                    ob[:, c0:c0 + csz], pout[:, 0:csz], b_sb[:, c0:c0 + csz]
                )
            nc.sync.dma_start(out=out_d[lt * 128:(lt + 1) * 128, :], in_=ob)

        # ---- attention + interleaved projection ----
        pending = None
        for ic in range(2):
            isl = slice(ic * 512, (ic + 1) * 512)
            for p in range(6):
                pu = psum.tile([128, 512], FP32, tag="sm", name="pu")
                nc.tensor.matmul(
                    pu, m2_sb[:, p * 128:(p + 1) * 128], tq2_sb[:, isl],
                    start=True, stop=True,
                )
                ub = (ic * 6 + p) * 2
                us = [u_sb[ub % 4], u_sb[(ub + 1) % 4]]
                for hh in range(2):
                    nc.vector.tensor_copy(us[hh][0:64, :], pu[64 * hh:64 * hh + 64, :])
                if ic == 1 and 1 <= p <= 4:
                    if pending is not None:
                        _emit_norm(*pending)
                        pending = None
                    _emit_proj(p - 1)
                for hh in range(2):
                    h = 2 * p + hh
                    kk, half = h // 2, (h % 2) * 64
                    if pending is not None:
                        _emit_norm(*pending)
                        pending = None
                    po = psum.tile([128, 512], FP32, tag="po", name="po")
                    for jp in range(4):
                        ps = psum.tile([128, 1024], FP32, tag="big", name="ps")
                        for t in range(2):
                            jt = 2 * jp + t
                            nc.tensor.matmul(
                                ps[:, t * 512:(t + 1) * 512],
                                tk_sb[:, jt * 128:(jt + 1) * 128], us[hh],
                                start=True, stop=True,
                            )
                        pt = work.tile([128, 1024], FP32R, tag="pt")
                        nc.scalar.activation(out=pt, in_=ps, func=EXP,
                                             bias=0.0, scale=1.0)
                        for t in range(2):
                            jt = 2 * jp + t
                            nc.tensor.matmul(
                                po[0:65, :], v_sb[jt][:, h, :],
                                pt[:, t * 512:(t + 1) * 512],
                                start=(jt == 0), stop=(jt == 7),
                            )
                    # copy po out fast (frees the PSUM bank); recip now,
                    # pz broadcast matmul + final mul deferred one head
                    oc = ocp.tile([128, 512], FP32, tag="oc", name="oc")
                    nc.vector.tensor_copy(oc[half:half + 64, :], po[0:64, :])
                    zrow = zp.tile([1, 512], FP32, tag="zrow", name="zrow")
                    nc.vector.tensor_copy(zrow, po[64:65, :])
                    rzt = zp.tile([1, 512], FP32, tag="rzt", name="rzt")
                    nc.vector.reciprocal_approx_fast(out=rzt, in_=zrow)
                    rz = rz_sb[h % 2]
                    nc.vector.tensor_copy(rz[0:1, :], rzt)
                    pending = (oc, rz, kk, half, isl)
        if pending is not None:
            _emit_norm(*pending)
            pending = None
        for lt in range(4, 8):
            _emit_proj(lt)

    nc.finalize()
    return nc


def _prep_shared(inputs):
    def comb(W1, W2):
        return np.ascontiguousarray(
            (np.asarray(W1, np.float32)[:, None, :]
             * np.asarray(W2, np.float32)[None, :, :]).reshape(DIM, R)
        )

    Aq = comb(inputs["W_Q1"], inputs["W_Q2"])
    Ak = comb(inputs["W_K1"], inputs["W_K2"])
    Av = comb(inputs["W_V1"], inputs["W_V2"])
    a_qk = np.concatenate([Aq, Ak], axis=1)  # [768, 128]
    a_qk_r = np.ascontiguousarray(
        a_qk.reshape(6, 128, 128).transpose(1, 0, 2).reshape(128, 768)
    )
    av_pad = np.zeros((DIM, 128), np.float32)
    av_pad[:, 0:R] = Av
    a_v_r = np.ascontiguousarray(
        av_pad.reshape(6, 128, 128).transpose(1, 0, 2).reshape(128, 768)
    )
    W_Q0 = np.asarray(inputs["W_Q0"], np.float32)
    W_K0 = np.asarray(inputs["W_K0"], np.float32)
    W_V0 = np.asarray(inputs["W_V0"], np.float32)
    scale = HD ** -0.5
    m2 = np.zeros((128, 768), np.float32)
    for h in range(H):
        sl = slice(h * HD, (h + 1) * HD)
        M = scale * (W_Q0[sl, :].T @ W_K0[sl, :])
        pp, half = h // 2, (h % 2) * 64
        m2[half:half + 64, pp * 128 + half:pp * 128 + half + 64] = M
    bv = np.ascontiguousarray(W_V0.T)  # [64, 768]
    pwt = np.ascontiguousarray(np.asarray(inputs["proj_w"], np.float32).T)
    bias = np.asarray(inputs["proj_b"], np.float32)
    zeros = np.zeros((128, 1024), np.float32)
    onesv = np.ones((128, H, 1), np.float32)
    onescol = np.zeros((128, 64), np.float32)
    onescol[0, :] = 1.0
    return dict(a_qk=a_qk_r, a_v=a_v_r, m2=m2, bv=bv, pwt=pwt, bias=bias,
                zeros=zeros, onesv=onesv, onescol=onescol)


def kernel(**inputs) -> np.ndarray:
    global LAST_EXEC_NS
    x = np.asarray(inputs["x"], np.float32)
    shared = _prep_shared(inputs)
    in_maps = []
    for b in range(B):
        m = dict(shared)
        m["xt"] = np.ascontiguousarray(x[b].T)
        in_maps.append(m)

    nc = _build_nc()
    trace = os.environ.get("KERNEL_TRACE", "0") == "1"
    res = run_bass_kernel_spmd(nc, in_maps, core_ids=list(range(NCORES)),
                               trace=trace)
    LAST_EXEC_NS = res.exec_time_ns
    out = np.stack([res.results[i]["out"] for i in range(NCORES)], axis=0)
    return out.astype(np.float32)
